# revision 17
# baseline (speedup 1.0000x reference)
"""Tropical (max-plus) linear kernel for Trainium2.

out[b, o] = max_i (W[o, i] + x[b, i]),  x: [512, 1024] f32, W: [512, 1024] f32.

Sharding (8 NeuronCores, SPMD): OUT across cores -> 64 output columns per
core; full x on every core; host concatenates core outputs on axis 1.

Execution on this axon-proxied setup is dominated by per-instruction
overhead (tens of us per instruction, nearly independent of its size), so
the kernel minimizes instruction count with wide multi-dim access patterns:

  host packs x as xh[p, bb*1024 + i] = f16(x[bb*128 + p, i])   [128, 4096]
  per block of G=8 output rows (8 blocks per core):
    1 DMA     : wb[128, G*1024] <- wh[g0:g0+G, :] partition-broadcast
    1 TT add  : s[128, G*4096] = xh (0-stride repeat over G)
                               + wb (0-stride repeat over the 4 b-blocks)
    1 reduce  : octile cols <- max over innermost 1024 of s[128, G, 4, 1024]
  4 output DMAs at the end.

~30 instructions total per core. All ops are first-class BIR instructions
(TENSOR_TENSOR_REDUCE raw-ISA faults on this toolchain; avoided).
"""

import numpy as np

import concourse.bacc as bacc
import concourse.tile as tile
from concourse import mybir
from concourse.bass_utils import run_bass_kernel_spmd

B, IN, OUT = 512, 1024, 512
NCORES = 8
O_PER_CORE = OUT // NCORES  # 64
NBB = B // 128  # 4 b-blocks
G = 8  # output rows per block
NBLK = O_PER_CORE // G  # 8 blocks

F32 = mybir.dt.float32
F16 = mybir.dt.float16
ADD = mybir.AluOpType.add
MAX = mybir.AluOpType.max
X_AX = mybir.AxisListType.X

import concourse.bass as bass


def _insert_zero_dim(ap: bass.AP, pos: int, reps: int) -> bass.AP:
    """Insert a 0-stride free dim (re-read broadcast) at free-dim index pos."""
    new = list(ap.ap)
    new.insert(1 + pos, [0, reps])
    return bass.AP(tensor=ap.tensor, offset=ap.offset, ap=new)


def build_nc(nrep: int = 1) -> bacc.Bacc:
    nc = bacc.Bacc("TRN2", num_devices=NCORES)
    xh = nc.dram_tensor("xh", [128, NBB * IN], F16, kind="ExternalInput")
    wh = nc.dram_tensor("wh", [O_PER_CORE, IN], F16, kind="ExternalInput")
    out = nc.dram_tensor("out", [B, O_PER_CORE], F32, kind="ExternalOutput")

    with tile.TileContext(nc) as tc:
        with (
            tc.tile_pool(name="big", bufs=1) as big,
            tc.tile_pool(name="wbp", bufs=2) as wbp,
            tc.tile_pool(name="sp", bufs=2) as sp,
        ):
            xt = big.tile([128, NBB * IN], F16, tag="xt", name="xt")
            nc.sync.dma_start(out=xt, in_=xh[:, :])
            octile = big.tile([128, NBB * O_PER_CORE], F32, tag="oc", name="oc")

            for r in range(nrep):
                for g in range(NBLK):
                    o0 = g * G
                    wb = wbp.tile([128, G * IN], F16, tag="wb", name="wb")
                    nc.sync.dma_start(
                        out=wb, in_=wh[o0 : o0 + G, :].partition_broadcast(128)
                    )
                    s = sp.tile([128, G * NBB * IN], F16, tag="s", name="s")
                    # in0: xh viewed [p, bb, i] + 0-stride G dim -> [p, G, 4, 1024]
                    in0 = _insert_zero_dim(
                        xt[:, :].rearrange("p (r i) -> p r i", r=NBB), 0, G
                    )
                    # in1: wb viewed [p, g, i] + 0-stride bb dim -> [p, G, 4, 1024]
                    in1 = _insert_zero_dim(
                        wb[:, :].rearrange("p (g i) -> p g i", g=G), 1, NBB
                    )
                    s4d = s[:, :].rearrange("p (g r i) -> p g r i", g=G, r=NBB)
                    nc.vector.tensor_tensor(s4d, in0, in1, op=ADD)
                    # cols: octile[p, bb*64 + (o0+g)] as [p, G, NBB]
                    cols = octile[:, :].rearrange("p (r o) -> p o r", r=NBB)[
                        :, o0 : o0 + G, :
                    ]
                    nc.vector.tensor_reduce(cols, s4d, axis=X_AX, op=MAX)

            for bb in range(NBB):
                nc.sync.dma_start(
                    out=out[bb * 128 : (bb + 1) * 128, :],
                    in_=octile[:, bb * O_PER_CORE : (bb + 1) * O_PER_CORE],
                )

    nc.compile()
    return nc


_NC = None


def _get_nc():
    global _NC
    if _NC is None:
        _NC = build_nc()
    return _NC


def make_in_maps(x: np.ndarray, W: np.ndarray):
    x = np.asarray(x, dtype=np.float32)
    W = np.asarray(W, dtype=np.float32)
    # xh[p, bb*1024+i] = x[bb*128+p, i]
    xh = np.ascontiguousarray(
        x.reshape(NBB, 128, IN).transpose(1, 0, 2).reshape(128, NBB * IN)
    ).astype(np.float16)
    return [
        {
            "xh": xh,
            "wh": np.ascontiguousarray(
                W[k * O_PER_CORE : (k + 1) * O_PER_CORE].astype(np.float16)
            ),
        }
        for k in range(NCORES)
    ]


def kernel(x, W, trace: bool = False):
    nc = _get_nc()
    res = run_bass_kernel_spmd(
        nc, make_in_maps(x, W), core_ids=list(range(NCORES)), trace=trace
    )
    out = np.concatenate([res.results[k]["out"] for k in range(NCORES)], axis=1)
    if trace:
        return out, res
    return out


# revision 20
# speedup vs baseline: 2.8226x; 2.8226x over previous
"""Tropical (max-plus) linear kernel for Trainium2.

out[b, o] = max_i (W[o, i] + x[b, i]),  x: [512, 1024] f32, W: [512, 1024] f32.

Sharding (8 NeuronCores, SPMD): OUT across cores -> 64 output columns per
core; full x on every core; host concatenates core outputs on axis 1.

Execution on this axon-proxied setup is dominated by per-instruction
overhead (tens of us per instruction, nearly independent of its size), so
the kernel minimizes instruction count with wide multi-dim access patterns:

  host packs x as xh[p, bb*1024 + i] = f16(x[bb*128 + p, i])   [128, 4096]
  per block of G=8 output rows (8 blocks per core):
    1 DMA     : wb[128, G*1024] <- wh[g0:g0+G, :] partition-broadcast
    1 TT add  : s[128, G*4096] = xh (0-stride repeat over G)
                               + wb (0-stride repeat over the 4 b-blocks)
    1 reduce  : octile cols <- max over innermost 1024 of s[128, G, 4, 1024]
  4 output DMAs at the end.

~30 instructions total per core. All ops are first-class BIR instructions
(TENSOR_TENSOR_REDUCE raw-ISA faults on this toolchain; avoided).
"""

import numpy as np

import concourse.bacc as bacc
import concourse.tile as tile
from concourse import mybir
from concourse.bass_utils import run_bass_kernel_spmd

B, IN, OUT = 512, 1024, 512
NCORES = 8
O_PER_CORE = OUT // NCORES  # 64
NBB = B // 128  # 4 b-blocks
G = 16  # output rows per block
NBLK = O_PER_CORE // G  # blocks per core

F32 = mybir.dt.float32
F16 = mybir.dt.float16
ADD = mybir.AluOpType.add
MAX = mybir.AluOpType.max
X_AX = mybir.AxisListType.X

import concourse.bass as bass


def _insert_zero_dim(ap: bass.AP, pos: int, reps: int) -> bass.AP:
    """Insert a 0-stride free dim (re-read broadcast) at free-dim index pos."""
    new = list(ap.ap)
    new.insert(1 + pos, [0, reps])
    return bass.AP(tensor=ap.tensor, offset=ap.offset, ap=new)


def build_nc(nrep: int = 1) -> bacc.Bacc:
    nc = bacc.Bacc("TRN2", num_devices=NCORES)
    xh = nc.dram_tensor("xh", [128, NBB * IN], F16, kind="ExternalInput")
    wh = nc.dram_tensor("wh", [O_PER_CORE, IN], F16, kind="ExternalInput")
    out = nc.dram_tensor("out", [B, O_PER_CORE], F32, kind="ExternalOutput")

    with tile.TileContext(nc) as tc:
        with (
            tc.tile_pool(name="big", bufs=1) as big,
            tc.tile_pool(name="wbp", bufs=1) as wbp,
            tc.tile_pool(name="sp", bufs=1) as sp,
        ):
            xt = big.tile([128, NBB * IN], F16, tag="xt", name="xt")
            nc.sync.dma_start(out=xt, in_=xh[:, :])
            octile = big.tile([128, NBB * O_PER_CORE], F32, tag="oc", name="oc")

            for r in range(nrep):
                for g in range(NBLK):
                    o0 = g * G
                    wb = wbp.tile([128, G * IN], F16, tag="wb", name="wb")
                    nc.sync.dma_start(
                        out=wb, in_=wh[o0 : o0 + G, :].partition_broadcast(128)
                    )
                    # Pad each 1024-chunk by 4 so the 4D access pattern cannot
                    # be flattened into a single >=2^16-count dim (16-bit
                    # num_elem ISA field).
                    PAD = 4
                    s = sp.tile([128, G * NBB * (IN + PAD)], F16, tag="s", name="s")
                    # in0: xh viewed [p, bb, i] + 0-stride G dim -> [p, G, 4, 1024]
                    in0 = _insert_zero_dim(
                        xt[:, :].rearrange("p (r i) -> p r i", r=NBB), 0, G
                    )
                    # in1: wb viewed [p, g, i] + 0-stride bb dim -> [p, G, 4, 1024]
                    in1 = _insert_zero_dim(
                        wb[:, :].rearrange("p (g i) -> p g i", g=G), 1, NBB
                    )
                    s4d = s[:, :].rearrange(
                        "p (g r j) -> p g r j", g=G, r=NBB
                    )[:, :, :, 0:IN]
                    nc.vector.tensor_tensor(s4d, in0, in1, op=ADD)
                    # cols: octile[p, bb*64 + (o0+g)] as [p, G, NBB]
                    cols = octile[:, :].rearrange("p (r o) -> p o r", r=NBB)[
                        :, o0 : o0 + G, :
                    ]
                    nc.vector.tensor_reduce(cols, s4d, axis=X_AX, op=MAX)

            for bb in range(NBB):
                nc.sync.dma_start(
                    out=out[bb * 128 : (bb + 1) * 128, :],
                    in_=octile[:, bb * O_PER_CORE : (bb + 1) * O_PER_CORE],
                )

    nc.compile()
    return nc


_NC = None


def _get_nc():
    global _NC
    if _NC is None:
        _NC = build_nc()
    return _NC


def make_in_maps(x: np.ndarray, W: np.ndarray):
    x = np.asarray(x, dtype=np.float32)
    W = np.asarray(W, dtype=np.float32)
    # xh[p, bb*1024+i] = x[bb*128+p, i]
    xh = np.ascontiguousarray(
        x.reshape(NBB, 128, IN).transpose(1, 0, 2).reshape(128, NBB * IN)
    ).astype(np.float16)
    return [
        {
            "xh": xh,
            "wh": np.ascontiguousarray(
                W[k * O_PER_CORE : (k + 1) * O_PER_CORE].astype(np.float16)
            ),
        }
        for k in range(NCORES)
    ]


def kernel(x, W, trace: bool = False):
    nc = _get_nc()
    res = run_bass_kernel_spmd(
        nc, make_in_maps(x, W), core_ids=list(range(NCORES)), trace=trace
    )
    out = np.concatenate([res.results[k]["out"] for k in range(NCORES)], axis=1)
    if trace:
        return out, res
    return out
